# revision 30
# baseline (speedup 1.0000x reference)
"""Distributed single-head attention block for trn2 (8 NeuronCores), v21.

reference:
    q = x @ Wq.T + bq ; k = x @ Wk.T + bk ; v = x @ Wv.T + bv
    out = x + softmax(q @ k.T / sqrt(D)) @ v       x: [4, 2048, 1024]

Sharding: 8 cores = 4 batches x 2 halves. Core c owns batch c//2 and
queries [h*1024, (h+1)*1024) with h = c%2. Everything else the core
needs is a plain host input — NO collectives at all:

  scores = Q.Kt = Xq.(Wq.T@Wk).Xt   (wqk = Wq.T@Wk host-folded, so the
                                     K projection and exchange vanish)
  attn   = softmax(scores) @ V = (P.X).Wv.T
                                    (V = X.Wv.T reassociated, so the V
                                     projection and exchange vanish too)

All PE operands are fp8 (TRN E4M3) running MatmulPerfMode.DoubleRow
(2 contraction chunks per instruction, 2x PE throughput). Four matmul
groups, 384 DoubleRow matmuls total, ~83us PE at 2.37GHz:
  1. qwkT proj: qwkT[e,q] = wqk.T @ Xq         (64 mm, psum->fp8 +bias)
  2. scoresT:   sT[k,q]   = Xt_chunk.T @ qwkT  (128 mm, exp -> PT fp8)
  3. YT:        Y.T[d,q]  = Xn_chunk.T @ PT    (128 mm, x recip -> fp8)
  4. attn:      out[q,e]  = YnT_chunk.T @ wvT  (64 mm, psum->bf16 out)

Scores are computed TRANSPOSED (keys on partitions) which kills the
128 PE transposes v19 needed; softmax denominators are computed on
otherwise-idle engines instead: a 16->1 pairwise add tree over PT's
key chunks on DVE (unit-stride), then gpsimd.partition_all_reduce
(Q7 daisy chain, ~3.5us) to sum the 128 key partitions — output
replicated on all partitions, exactly the shape the Yn normalization
multiply needs as its in1. Each half's den pipeline is emitted right
after that half's exps so it hides under the NEXT phase's matmuls
(v20 emitted den(1) after yt(0) and the PE stalled 4us on the psum
ring behind tree->allreduce->reciprocal). reciprocal_approx_fast
(~51 ULP) replaces reciprocal (3.4us -> 0.7us); den is in [76, 216]
so its undefined edge cases are unreachable. Softmax is invariant to
P's scale so exp gets bias=-4 (measured score max 8.33; e4m3 Inf at
240 would poison everything).

Host pre-scales wqk and wvT by 16 (lifts weight mass out of e4m3
subnormals): the wqk x16 is folded into the exp scale (1/512), the
wvT x16 into the host-side output /16. Measured input stats (fixed
rng key): |x|<5.2, P<64, |Yn|<1, den in [76, 216]. Emulated
end-to-end fp8 numerics: rel_l2 6.65e-3 (gate 2e-2).

Every input is host-shuffled into its EXACT SBUF layout ([128 parts,
free...] row-major) so each load is one DMA of 128 contiguous 4-16KB
rows — v20's strided loads cost up to 3.7us of SWDGE descriptor
generation per issue on the queue head. All pools open in ONE scope:
v20's nested scopes emitted all-engine barriers that held the first
input DMA until the warmup drained (~1.7us dead at the start). A
dma_start still costs ~0.7us of issue time on the triggering queue:
    sync   : wqk (3 pieces, first compute), xT-lo, xN, output stores
             (idle by then; v20 serialized stores behind ScalarE
             epilogues and paid ~1.2us extra drain)
    scalar : bq + xqT-lo, wv; exp activations; half the output casts
    vector : qwkT bias-adds, den trees, recips, Yn muls, half the
             output casts
    gpsimd : xqT-hi, xT-hi + the two partition_all_reduce calls
"""

import numpy as np

B, S, D = 4, 2048, 1024
SQ = S // 2  # queries owned per core
NCORES = 8
DC = D // 128  # contraction chunks over embed
EC = D // 128  # embed chunks
SC = S // 128  # key chunks, full batch
QT = SQ // 128  # query tiles per core
QH = SQ // 512  # query 512-halves per core
EJ = D // 512  # 512-wide embed column chunks

WSCALE = 16.0  # host weight pre-scale (fp8 subnormal headroom)
EXP_BIAS = -4.0  # P = exp(s/sqrt(D) + bias); cancels in softmax

_cache = {}


def _build():
    import concourse.bass as bass
    import concourse.tile as tile
    from concourse import bacc, bass_isa, mybir

    f32 = mybir.dt.float32
    bf16 = mybir.dt.bfloat16
    fp8 = mybir.dt.float8e4
    Alu = mybir.AluOpType
    Act = mybir.ActivationFunctionType
    DR = mybir.MatmulPerfMode.DoubleRow

    nc = bacc.Bacc(None, target_bir_lowering=False, debug=False)

    # Inputs are pre-shuffled on the host into the exact SBUF layouts.
    xqT_d = nc.declare_dram_parameter("xqT", [128, QH, DC, 512], fp8, isOutput=False)
    xT_d = nc.declare_dram_parameter("xT", [128, DC, S], fp8, isOutput=False)
    xN_d = nc.declare_dram_parameter("xN", [128, SC, D], fp8, isOutput=False)
    wqk_d = nc.declare_dram_parameter("wqkE", [128, EC, DC, 128], fp8, isOutput=False)
    wv_d = nc.declare_dram_parameter("wvT", [128, DC, D], fp8, isOutput=False)
    bq_d = nc.declare_dram_parameter("bq", [128, EC], f32, isOutput=False)
    out_d = nc.declare_dram_parameter("out", [SQ, D], bf16, isOutput=True)

    with tile.TileContext(nc) as tc:
        with (
            tc.tile_pool(name="pers", bufs=1) as pers,
            tc.tile_pool(name="den", bufs=2) as denp,
            tc.tile_pool(name="ot", bufs=3) as otp,
            tc.tile_pool(name="proj_ps", bufs=4, space="PSUM") as proj_ps,
            tc.tile_pool(name="mm_ps", bufs=4, space="PSUM") as mm_ps,
        ):
            xqT_sb = pers.tile([128, QH, DC, 512], fp8, tag="xqT")
            xT_sb = pers.tile([128, DC, S], fp8, tag="xT")
            xN_sb = pers.tile([128, SC, D], fp8, tag="xN")
            wqk_sb = pers.tile([128, EC, DC, 128], fp8, tag="wqk")
            wv_sb = pers.tile([128, DC, D], fp8, tag="wv")
            qwkT_sb = pers.tile([128, DC, SQ], fp8, tag="qwkT")
            PT_sb = pers.tile([128, QH, SC, 512], fp8, tag="PT")
            YnT_sb = pers.tile([128, DC, SQ], fp8, tag="YnT")
            recip_sb = pers.tile([128, SQ], f32, tag="recip")
            bq_sb = pers.tile([128, EC], f32, tag="bq")
            ebias = pers.tile([128, 1], f32, tag="ebias")
            warm_sb = pers.tile([128, 512], bf16, tag="warm")
            warm_dump = pers.tile([128, 512], f32, tag="warm_dump")

            # ---- input DMAs first: nothing upstream can block them ----
            def flat_piece(q, dst, src_d, lo, hi, inner):
                """Load dst[:, lo:hi, ...] from the identically-laid-out
                dram tensor: 128 contiguous rows of (hi-lo)*inner bytes."""
                n = src_d.shape[1]
                q.dma_start(
                    out=dst[:, lo:hi],
                    in_=bass.AP(
                        tensor=src_d.ap().tensor,
                        offset=lo * inner,
                        ap=[[n * inner, 128], [1, (hi - lo) * inner]],
                    ),
                )

            # Need-ordered across the three dma-capable queues. Measured:
            # per-ring ~40-100 GB/s, aggregate ~320 GB/s shared — the 7MB
            # of inputs is a ~22us wall, so the start-critical bytes (xqT
            # by query-half + wqk[0]) must monopolize the ring heads and
            # the bulk (xT -> xN -> wv, by need time: ~24/51/79us) queues
            # strictly behind them.
            # The scalar ring (q10) measures 2x slower than sync/gpsimd
            # under early contention — start-critical bytes ride sync +
            # gpsimd only; the scalar ring gets the late-need bulk.
            flat_piece(nc.sync, wqk_sb, wqk_d, 0, 1, DC * 128)
            nc.scalar.dma_start(out=bq_sb, in_=bq_d.ap())
            flat_piece(nc.sync, xqT_sb, xqT_d, 0, 1, DC * 512)
            flat_piece(nc.gpsimd, xqT_sb, xqT_d, 1, QH, DC * 512)
            flat_piece(nc.sync, wqk_sb, wqk_d, 1, EC, DC * 128)
            flat_piece(nc.sync, xT_sb, xT_d, 0, DC // 2, S)
            flat_piece(nc.gpsimd, xT_sb, xT_d, DC // 2, DC, S)
            flat_piece(nc.scalar, xN_sb, xN_d, 0, SC // 2, D)
            flat_piece(nc.scalar, xN_sb, xN_d, SC // 2, SC, D)
            flat_piece(nc.scalar, wv_sb, wv_d, 0, DC, D)

            nc.vector.memset(ebias, EXP_BIAS)
            nc.vector.memset(warm_sb, 0.0)

            # PE warmup: dense dummy matmuls while the first input DMAs
            # land, so the HAM clock gate is already ramped when real work
            # starts. Rides the proj_ps ring (no extra PSUM pool barrier).
            wps = proj_ps.tile([128, 512], f32, tag="ps")
            NWARM = 14
            for i in range(NWARM):
                nc.tensor.matmul(
                    wps,
                    lhsT=warm_sb[:, 0:128],
                    rhs=warm_sb,
                    start=(i == 0),
                    stop=(i == NWARM - 1),
                )
            nc.vector.tensor_copy(out=warm_dump, in_=wps)

            # ---- 1. qwkT projection: qwkT[e,q] = wqk.T @ Xq + bqk ----
            for dc in range(DC):
                for j in range(QH):
                    jsl = slice(j * 512, (j + 1) * 512)
                    ps = proj_ps.tile([128, 512], f32, tag="ps")
                    for ic in range(0, DC, 2):
                        nc.tensor.matmul(
                            ps,
                            lhsT=wqk_sb[:, dc, ic : ic + 2, :],
                            rhs=xqT_sb[:, j, ic : ic + 2, :],
                            start=(ic == 0),
                            stop=(ic == DC - 2),
                            perf_mode=DR,
                        )
                    nc.vector.tensor_scalar_add(
                        out=qwkT_sb[:, dc, jsl],
                        in0=ps,
                        scalar1=bq_sb[:, dc : dc + 1],
                    )

            # wqk is host-scaled by WSCALE; exp scale removes it together
            # with the softmax 1/sqrt(D).
            exp_scale = float(1.0 / (WSCALE * np.sqrt(D)))

            # ---- 2+3. per query-half: scoresT+exp, den, then YT ----
            def emit_scores_half(qh):
                qsl = slice(qh * 512, (qh + 1) * 512)
                for kt in range(SC):
                    ps = mm_ps.tile([128, 512], f32, tag="mm")
                    for dc in range(0, DC, 2):
                        nc.tensor.matmul(
                            ps,
                            lhsT=xT_sb[:, dc : dc + 2, kt * 128 : (kt + 1) * 128],
                            rhs=qwkT_sb[:, dc : dc + 2, qsl],
                            start=(dc == 0),
                            stop=(dc == DC - 2),
                            perf_mode=DR,
                        )
                    nc.scalar.activation(
                        out=PT_sb[:, qh, kt, :],
                        in_=ps,
                        func=Act.Exp,
                        scale=exp_scale,
                        bias=ebias,
                    )

            def emit_den_half(qh):
                # 16->1 pairwise add tree over key chunks (unit stride on
                # DVE), then Q7 daisy-chain sum over the 128 key partitions;
                # result lands replicated on every partition, which is
                # exactly what the Yn multiply needs.
                qsl = slice(qh * 512, (qh + 1) * 512)
                dtA = denp.tile([128, 8, 512], bf16, tag="dtA")
                dtB = denp.tile([128, 4, 512], bf16, tag="dtB")
                dtC = denp.tile([128, 2, 512], bf16, tag="dtC")
                dden = denp.tile([128, 512], bf16, tag="dden")
                dall = denp.tile([128, 512], bf16, tag="dall")
                dallf = denp.tile([128, 512], f32, tag="dallf")
                nc.vector.tensor_tensor(
                    out=dtA,
                    in0=PT_sb[:, qh, 0:8, :],
                    in1=PT_sb[:, qh, 8:16, :],
                    op=Alu.add,
                )
                nc.vector.tensor_tensor(
                    out=dtB, in0=dtA[:, 0:4, :], in1=dtA[:, 4:8, :], op=Alu.add
                )
                nc.vector.tensor_tensor(
                    out=dtC, in0=dtB[:, 0:2, :], in1=dtB[:, 2:4, :], op=Alu.add
                )
                nc.vector.tensor_tensor(
                    out=dden, in0=dtC[:, 0, :], in1=dtC[:, 1, :], op=Alu.add
                )
                nc.gpsimd.partition_all_reduce(
                    out_ap=dall,
                    in_ap=dden,
                    channels=128,
                    reduce_op=bass_isa.ReduceOp.add,
                )
                # den in [76, 216]: approx-reciprocal edge cases unreachable
                # (needs f32 in; the all-reduce stays bf16 to halve its
                # daisy-chain bytes)
                nc.vector.tensor_copy(out=dallf, in_=dall)
                nc.vector.reciprocal_approx_fast(recip_sb[:, qsl], dallf)

            def emit_yt_half(qh):
                qsl = slice(qh * 512, (qh + 1) * 512)
                for dc in range(DC):
                    ps = mm_ps.tile([128, 512], f32, tag="mm")
                    for kt in range(0, SC, 2):
                        nc.tensor.matmul(
                            ps,
                            lhsT=xN_sb[:, kt : kt + 2, dc * 128 : (dc + 1) * 128],
                            rhs=PT_sb[:, qh, kt : kt + 2, :],
                            start=(kt == 0),
                            stop=(kt == SC - 2),
                            perf_mode=DR,
                        )
                    nc.vector.tensor_tensor(
                        out=YnT_sb[:, dc, qsl],
                        in0=ps,
                        in1=recip_sb[:, qsl],
                        op=Alu.mult,
                    )

            emit_scores_half(0)
            emit_den_half(0)
            emit_scores_half(1)
            emit_den_half(1)
            emit_yt_half(0)
            emit_yt_half(1)

            # ---- 4. attn: out[q,e] = YnT.T @ wvT; psum -> bf16 out ----
            for qt in range(QT):
                qsl = slice(qt * 128, (qt + 1) * 128)
                ot = otp.tile([128, D], bf16, tag="ot")
                for j2 in range(EJ):
                    jsl = slice(j2 * 512, (j2 + 1) * 512)
                    pa = mm_ps.tile([128, 512], f32, tag="mm")
                    for dc in range(0, DC, 2):
                        nc.tensor.matmul(
                            pa,
                            lhsT=YnT_sb[:, dc : dc + 2, qsl],
                            rhs=wv_sb[:, dc : dc + 2, jsl],
                            start=(dc == 0),
                            stop=(dc == DC - 2),
                            perf_mode=DR,
                        )
                    # epilogue cast split across DVE and ScalarE; stores
                    # split across the sync and scalar QUEUES so the 2MB of
                    # output transfers don't serialize on one dma ring at
                    # the tail.
                    if j2 == 0:
                        nc.vector.tensor_copy(out=ot[:, jsl], in_=pa)
                        nc.sync.dma_start(out=out_d[qsl, jsl], in_=ot[:, jsl])
                    else:
                        nc.scalar.activation(out=ot[:, jsl], in_=pa, func=Act.Copy)
                        nc.scalar.dma_start(out=out_d[qsl, jsl], in_=ot[:, jsl])

    nc.compile()
    return nc


def _get_nc():
    if "nc" not in _cache:
        _cache["nc"] = _build()
    return _cache["nc"]


def kernel(embedded, Wq, bq, Wk, bk, Wv, bv):
    import ml_dtypes

    from concourse.bass_utils import run_bass_kernel_spmd

    fp8 = ml_dtypes.float8_e4m3  # TRN E4M3: max 240, Inf beyond

    def q8(a):
        return np.clip(np.asarray(a, dtype=np.float32), -240.0, 240.0).astype(fp8)

    x = np.ascontiguousarray(np.asarray(embedded, dtype=np.float32))
    Wq = np.asarray(Wq, dtype=np.float32)
    Wk = np.asarray(Wk, dtype=np.float32)
    Wv = np.asarray(Wv, dtype=np.float32)
    bq = np.ascontiguousarray(np.asarray(bq, dtype=np.float32))
    bv = np.ascontiguousarray(np.asarray(bv, dtype=np.float32))

    C = np.ascontiguousarray

    # scores = Q.Kt = Xq @ (Wq.T @ Wk) @ Xt + (bq @ Wk) @ Xt: weights-only
    # constants, computed on the host in f32, pre-scaled by WSCALE for fp8.
    wqwk = q8((Wq.T @ Wk).astype(np.float32) * WSCALE)
    # [128, EC, DC, 128]: w[p, ec, dc, j] = wqwk[dc*128+p, ec*128+j]
    # (contraction d on partitions, output echunk ec, output col j)
    wqkE = C(wqwk.reshape(DC, 128, EC, 128).transpose(1, 2, 0, 3))
    bqk = C(((bq @ Wk).astype(np.float32) * WSCALE).reshape(EC, 128).T)
    # [128, DC, D]: w[p, dc, e] = Wv.T[dc*128+p, e]
    wvT = C(q8(Wv.T * WSCALE).reshape(DC, 128, D).transpose(1, 0, 2))

    x8 = [q8(x[b]) for b in range(B)]
    # [128, DC, S]: xT[p, dc, s] = x[s, dc*128+p]
    xTh = [C(a.reshape(S, DC, 128).transpose(2, 1, 0)) for a in x8]
    # [128, SC, D]: xN[p, c, e] = x[c*128+p, e]
    xNh = [C(a.reshape(SC, 128, D).transpose(1, 0, 2)) for a in x8]

    in_maps = []
    for c in range(NCORES):
        b, h = c // 2, c % 2
        qs = slice(h * SQ, (h + 1) * SQ)
        # [128, QH, DC, 512]: xqT[p, qh, dc, qq] = x[h*SQ+qh*512+qq, dc*128+p]
        xqTh = C(
            x8[b][qs].reshape(QH, 512, DC, 128).transpose(3, 0, 2, 1)
        )
        in_maps.append(
            {
                "xqT": xqTh,
                "xT": xTh[b],
                "xN": xNh[b],
                "wqkE": wqkE,
                "wvT": wvT,
                "bq": bqk,
            }
        )

    _cache["in_maps"] = in_maps
    nc = _get_nc()
    res = run_bass_kernel_spmd(nc, in_maps, core_ids=list(range(NCORES)))
    out = np.empty((B, S, D), dtype=np.float32)
    for c in range(NCORES):
        b, h = c // 2, c % 2
        out[b, h * SQ : (h + 1) * SQ, :] = res.results[c]["out"].astype(np.float32)
    # device output is 16*attn (wvT host-scaled); undo here, then the
    # residual (+ V bias, which passes through the attention average)
    out *= 1.0 / WSCALE
    out += x + bv
    return out
